# revision 31
# baseline (speedup 1.0000x reference)
"""Masked-softmax attention pooling on 8 TRN2 NeuronCores.

Reference computation (per batch b):
    q = hidden @ W.T                      # [H]
    alphas[s] = eo[b, s, :] . q           # [S]
    alphas = where(mask, -1e16, alphas)
    scores = softmax(alphas)              # over S
    out[b] = sum_s scores[s] * eo[b, s, :]

Sharding: data-parallel over batch (8 batches/core), W replicated.
encoder_output dominates traffic (64 MiB/core) and is streamed from HBM
exactly once, interleaved 4 s-rows per partition so each DMA descriptor
moves 16 KiB contiguous (near line rate). Per 512-row chunk:
  - DVE scalar_tensor_tensor fuses multiply(+discarded bf16 out) with the
    f32 row-sum accumulate -> alphas
  - ScalarE casts the chunk to bf16 for the weighted sum
  - TensorE weighted sum in bf16 (score column stationary, eo moving),
    one PE pass per 512 columns
Softmax is two-level (per-partition max/sum, then a cross-partition
fix-up via TensorE transposes); 1/den is folded into the per-partition
fix-up factor g before it is broadcast back, so the final copy is plain.
"""

from contextlib import ExitStack

import numpy as np

import concourse.bass as bass
import concourse.tile as tile
from concourse import bacc, mybir
from concourse._compat import get_trn_type
from concourse.bass_utils import run_bass_kernel_spmd
from concourse.masks import make_identity

B, S, H = 64, 2048, 1024
N_CORES = 8
BL = B // N_CORES      # 8 batches per core
RP = 4                 # s-rows interleaved per partition (16 KiB descriptors)
NCH = S // (128 * RP)  # 4 DMA chunks per batch, 512 rows each
SC = S // 128          # 16 alphas columns (one per (chunk, t))
HC = H // 128
F32 = mybir.dt.float32
BF16 = mybir.dt.bfloat16

NEG_BIG = -1.0e16
EO_BUFS = 3            # f32 landing tiles [128, RP*H] (16 KiB/partition each)
XB_BUFS = 8            # bf16 tiles: batch b (4) + batch b+1 (4)


def _build(BL=BL, S=S, H=H):
    nc = bacc.Bacc(get_trn_type() or "TRN2", target_bir_lowering=False)

    hid_d = nc.dram_tensor("hidden", [BL, H], F32, kind="ExternalInput")
    eo_d = nc.dram_tensor("encoder_output", [BL, S, H], F32, kind="ExternalInput")
    # host-prepped additive mask: [128, BL, SC], -1e16 at masked positions,
    # column index col = chunk*RP + t  <->  s = chunk*512 + 4p + t
    mk_d = nc.dram_tensor("encoder_mask", [128, BL, SC], F32, kind="ExternalInput")
    w_d = nc.dram_tensor("W", [H, H], F32, kind="ExternalInput")
    out_d = nc.dram_tensor("out", [BL, H], F32, kind="ExternalOutput")

    AF = mybir.ActivationFunctionType
    ALU = mybir.AluOpType
    AX = mybir.AxisListType

    with tile.TileContext(nc) as tc, ExitStack() as ctx:
        sing = ctx.enter_context(tc.tile_pool(name="sing", bufs=1))
        wpool = ctx.enter_context(tc.tile_pool(name="wpool", bufs=2))
        eop = ctx.enter_context(tc.tile_pool(name="eop", bufs=EO_BUFS))
        xbp = ctx.enter_context(tc.tile_pool(name="xbp", bufs=XB_BUFS))
        trashp = ctx.enter_context(tc.tile_pool(name="trashp", bufs=1))
        smallp = ctx.enter_context(tc.tile_pool(name="smallp", bufs=2))
        outp = ctx.enter_context(tc.tile_pool(name="outp", bufs=1))
        qbp = ctx.enter_context(tc.tile_pool(name="qbp", bufs=3))
        ps_tr = ctx.enter_context(tc.tile_pool(name="ps_tr", bufs=4, space="PSUM"))
        ps_big = ctx.enter_context(tc.tile_pool(name="ps_big", bufs=2, space="PSUM"))
        dramp = ctx.enter_context(tc.tile_pool(name="dramp", bufs=1, space="DRAM"))

        # ---- constants + small inputs; W rows are the prologue's long pole,
        # so their DMAs go first on the sync queue
        ident = sing.tile([128, 128], F32)
        make_identity(nc, ident[:])
        # hidden rows 0/1 broadcast across partitions for the DVE q fast path
        hid0b = qbp.tile([128, H], F32, tag="qb")
        nc.gpsimd.dma_start(hid0b[:], hid_d[0:1, :].to_broadcast([128, H]))
        hid1b = qbp.tile([128, H], F32, tag="qb")
        nc.gpsimd.dma_start(hid1b[:], hid_d[1:2, :].to_broadcast([128, H]))

        # small input DMAs go before the W rows on the sync queue: hT (below)
        # is ahead of the W transposes on the TensorE FIFO, so hid must not
        # queue behind W-row DMAs whose buffer slots wait on those transposes
        mask_sb = sing.tile([128, BL, SC], F32)
        nc.sync.dma_start(mask_sb[:], mk_d[:])
        hid = sing.tile([BL, H], F32)
        nc.sync.dma_start(hid[:], hid_d[:])

        wrows = []
        wt = sing.tile([128, HC, H], F32)   # W^T, per h-chunk [128h, 1024o]
        q01T = sing.tile([128, HC, 2], F32)  # q rows 0/1, o on partitions
        for o in range(HC):
            wrow = wpool.tile([128, H], F32, tag="wrow", bufs=8)
            nc.sync.dma_start(wrow[:], w_d[bass.ts(o, 128), :])
            wrows.append(wrow)
            # q{0,1}[128o-chunk] = rowsum(wrow * hidden_b) on the (idle) DVE,
            # pipelined with the W DMAs -> batches 0/1 start ~30us earlier
            for bb, hb in ((0, hid0b), (1, hid1b)):
                tr01 = trashp.tile([128, H], BF16, tag="trash")
                nc.vector.scalar_tensor_tensor(
                    out=tr01[:], in0=wrow[:], scalar=1.0, in1=hb[:],
                    op0=ALU.mult, op1=ALU.mult,
                    accum_out=q01T[:, o, bb : bb + 1],
                )

        # q rows 0/1 -> DRAM (via a [HC, 128] transpose) for partition-broadcast.
        # Emitted first on the TensorE queue: the wrows have dedicated slots
        # (bufs=8) so nothing upstream can wait on later TensorE work.
        q_dram = dramp.tile([BL, H], F32)
        for bb in range(2):
            p = ps_tr.tile([HC, 128], F32, tag="tr")
            nc.tensor.transpose(p[:], q01T[:, :, bb], ident[:])
            qrow = outp.tile([HC, 128], F32, tag="qrow", bufs=2)
            nc.scalar.copy(qrow[:], p[:])
            nc.sync.dma_start(
                q_dram[bb : bb + 1, :].rearrange("one (c p) -> (one c) p", p=128),
                qrow[:],
            )

        hT = sing.tile([128, HC, BL], F32)  # hidden^T, per h-chunk [128h, BL]
        for h in range(HC):
            p = ps_tr.tile([128, BL], F32, tag="tr")
            nc.tensor.transpose(p[:], hid[:, bass.ts(h, 128)], ident[0:BL, 0:BL])
            nc.scalar.copy(hT[:, h, :], p[:])

        # transpose W o-chunks 0-3, run the nh=0 half of the q matmul while
        # o-chunks 4-7 transpose, then the nh=1 half. PSUM->SBUF copies
        # alternate between ScalarE and DVE to halve the copy chain.
        q_ps = ps_big.tile([BL, H], F32, tag="big")

        def _wt_chunk(o):
            wrow = wrows[o]
            for h in range(HC):
                p = ps_tr.tile([128, 128], F32, tag="tr")
                nc.tensor.transpose(p[:], wrow[:, bass.ts(h, 128)], ident[:])
                if h % 2 == 0:
                    nc.scalar.copy(wt[:, h, bass.ts(o, 128)], p[:])
                else:
                    nc.vector.tensor_copy(wt[:, h, bass.ts(o, 128)], p[:])

        def _q_half(nh):
            for h in range(HC):
                nc.tensor.matmul(
                    q_ps[:, bass.ts(nh, 512)],
                    hT[:, h, :],
                    wt[:, h, bass.ts(nh, 512)],
                    start=(h == 0),
                    stop=(h == HC - 1),
                )

        for o in range(4):
            _wt_chunk(o)
        _q_half(0)
        for o in range(4, HC):
            _wt_chunk(o)
        _q_half(1)
        q_sb = outp.tile([BL, H], F32, tag="c_sb")
        nc.scalar.copy(q_sb[:], q_ps[:])
        # rows 2-7 of q to DRAM (0/1 already written by the DVE fast path)
        nc.sync.dma_start(q_dram[2:BL, :], q_sb[2:BL, :])

        # ---- main loop over local batches, software-pipelined: batch b's
        # softmax + weighted sum are emitted BETWEEN batch b+1's chunk
        # pairs so the DVE queue never head-of-line blocks on the
        # cross-engine softmax chain
        state = {}

        def start_batch(b):
            qb = qbp.tile([128, H], F32, tag="qb")
            nc.gpsimd.dma_start(qb[:], q_dram[b : b + 1, :].to_broadcast([128, H]))
            alphas = smallp.tile([128, SC], F32, tag="alphas")
            state[b] = (qb, alphas, [])

        def emit_chunks(b, chunks):
            qb, alphas, xbs = state[b]
            # [NCH, 128, RP*H] view: 16 KiB contiguous per partition line
            eo_v = eo_d[b].rearrange("(c p t) h -> c p (t h)", p=128, t=RP)
            for c in chunks:
                x = eop.tile([128, RP, H], F32, tag="x")
                nc.sync.dma_start(x.rearrange("p t h -> p (t h)"), eo_v[c])
                xb = xbp.tile([128, RP, H], BF16, tag="xb")
                xbs.append(xb)
                for t in range(RP):
                    col = c * RP + t
                    # fused DVE: trash = (x*1)*qb (bf16, discarded);
                    # alphas[:, col] = f32 row-sum of the products
                    trash = trashp.tile([128, H], BF16, tag="trash")
                    nc.vector.scalar_tensor_tensor(
                        out=trash[:], in0=x[:, t, :], scalar=1.0, in1=qb[:],
                        op0=ALU.mult, op1=ALU.mult,
                        accum_out=alphas[:, col : col + 1],
                    )
                    # bf16 copy for the weighted sum
                    nc.scalar.copy(xb[:, t, :], x[:, t, :])

        def finish_batch(b):
            qb, alphas, xbs = state.pop(b)
            am = smallp.tile([128, SC], F32, tag="am")
            nc.vector.tensor_add(am[:], alphas[:], mask_sb[:, b, :])

            # two-level softmax. Row-level (per partition p over its 16 cols):
            #   nm[p] = -max_col am[p,col]
            #   e[p,col] = exp(am[p,col] + nm[p]);  s1[p] = sum_col e[p,col]
            # Cross-partition fix-up:
            #   mn = min_p nm[p]  (= -global max)
            #   g[p] = exp(-(nm[p] - mn)) / den;  den = sum_p s1[p]*exp(-(nm-mn))
            #   us[p,col] = e[p,col] * g[p]  (bf16, feeds TensorE directly)
            m1 = smallp.tile([128, 1], F32, tag="m1")
            nc.vector.tensor_reduce(
                out=m1[:], in_=am[:], axis=AX.X, op=ALU.max,
            )
            nm = smallp.tile([128, 1], F32, tag="nm")
            nc.scalar.mul(nm[:], m1[:], -1.0)
            e = smallp.tile([128, SC], F32, tag="e")
            s1 = smallp.tile([128, 1], F32, tag="s1")
            nc.scalar.activation(
                out=e[:], in_=am[:], func=AF.Exp,
                bias=nm[:], scale=1.0, accum_out=s1[:],
            )
            nmp = ps_tr.tile([1, 128], F32, tag="tr")
            nc.tensor.transpose(nmp[:], nm[:], ident[:])
            s1p = ps_tr.tile([1, 128], F32, tag="tr")
            nc.tensor.transpose(s1p[:], s1[:], ident[:])
            mn = smallp.tile([1, 1], F32, tag="mn")
            nc.vector.tensor_reduce(out=mn[:], in_=nmp[:], axis=AX.X, op=ALU.min)
            dn = smallp.tile([1, 128], F32, tag="dn")
            nc.vector.tensor_scalar_sub(dn[:], nmp[:], mn[0:1, 0:1])
            g = smallp.tile([1, 128], F32, tag="g")
            nc.scalar.activation(out=g[:], in_=dn[:], func=AF.Exp, bias=0.0, scale=-1.0)
            wtr = smallp.tile([1, 128], F32, tag="wtr")
            nc.vector.tensor_mul(wtr[:], s1p[:], g[:])
            den = smallp.tile([1, 1], F32, tag="den")
            nc.vector.tensor_reduce(
                out=den[:], in_=wtr[:], axis=AX.X, op=ALU.add,
            )
            r = smallp.tile([1, 1], F32, tag="r")
            nc.vector.reciprocal(r[:], den[:])
            gr = smallp.tile([1, 128], F32, tag="gr")
            nc.vector.tensor_scalar_mul(gr[:], g[:], r[0:1, 0:1])
            gp = ps_tr.tile([128, 1], F32, tag="tr")
            nc.tensor.transpose(gp[:], gr[:], ident[0:1, 0:1])
            us = smallp.tile([128, SC], BF16, tag="us")
            nc.vector.tensor_scalar_mul(us[:], e[:], gp[:, 0:1])

            # c = sum_s us[s] * eo[s, :]  via TensorE in bf16,
            # score column stationary, eo chunk moving
            c_ps = ps_big.tile([1, H], F32, tag="big")
            for c in range(NCH):
                for t in range(RP):
                    col = c * RP + t
                    for nh in range(H // 512):
                        nc.tensor.matmul(
                            c_ps[0:1, bass.ts(nh, 512)],
                            us[:, col : col + 1],
                            xbs[c][:, t, bass.ts(nh, 512)],
                            start=(col == 0),
                            stop=(col == SC - 1),
                        )
            c_sb = outp.tile([1, H], F32, tag="c_sb")
            nc.scalar.copy(c_sb[:], c_ps[:])
            nc.sync.dma_start(out_d[b : b + 1, :], c_sb[:])

        start_batch(0)
        emit_chunks(0, range(NCH))
        for b in range(BL):
            if b + 1 < BL:
                start_batch(b + 1)
                emit_chunks(b + 1, range(NCH // 2))
            finish_batch(b)
            if b + 1 < BL:
                emit_chunks(b + 1, range(NCH // 2, NCH))

    nc.compile()
    return nc


_CACHE = {}


def _get_nc():
    if "nc" not in _CACHE:
        _CACHE["nc"] = _build()
    return _CACHE["nc"]


def _make_in_maps(hidden, encoder_output, encoder_mask, W):
    hidden = np.ascontiguousarray(hidden, dtype=np.float32)
    eo = np.ascontiguousarray(encoder_output, dtype=np.float32)
    # additive mask in [128p, b, col] layout (col = chunk*RP + t,
    # s = chunk*512 + 4p + t): -1e16 at masked positions
    mk = encoder_mask.reshape(B, S).astype(np.float32) * np.float32(NEG_BIG)
    mk = np.ascontiguousarray(
        mk.reshape(B, NCH, 128, RP).transpose(2, 0, 1, 3).reshape(128, B, SC)
    )
    W = np.ascontiguousarray(W, dtype=np.float32)
    in_maps = []
    for i in range(N_CORES):
        sl = slice(i * BL, (i + 1) * BL)
        in_maps.append(
            {
                "hidden": hidden[sl],
                "encoder_output": eo[sl],
                "encoder_mask": np.ascontiguousarray(mk[:, sl, :]),
                "W": W,
            }
        )
    return in_maps


def run(hidden, encoder_output, encoder_mask, W, trace=False):
    nc = _get_nc()
    in_maps = _make_in_maps(hidden, encoder_output, encoder_mask, W)
    res = run_bass_kernel_spmd(nc, in_maps, list(range(N_CORES)), trace=trace)
    out = np.concatenate([res.results[i]["out"] for i in range(N_CORES)], axis=0)
    return out, res


def kernel(hidden, encoder_output, encoder_mask, W):
    out, _ = run(hidden, encoder_output, encoder_mask, W, trace=False)
    return out
